# revision 10
# baseline (speedup 1.0000x reference)
"""Trainium2 Bass kernel: DETR-style bipartite matching cost matrix.

cost[b, q, t] = L1(pred_box, tgt_box) - softmax(logits)[q, tgt_id[t]] - CIoU(pred_box, tgt_box)

Sharding: data-parallel over batch. Core i computes the full [Q, T] cost slab
for batch i; tgt tensors are replicated. Host stacks the 8 slabs.
"""

import math
from contextlib import ExitStack

import numpy as np

import concourse.bass as bass
import concourse.bacc as bacc
import concourse.mybir as mybir
import concourse.tile as tile
from concourse.bass_utils import run_bass_kernel_spmd
from concourse.masks import make_identity

B, Q, C, T = 8, 900, 92, 1600
EPS = 1e-6
P = 128
NQT = (Q + P - 1) // P  # 8 query tiles; last is ragged (4 rows)
F32 = mybir.dt.float32
I32 = mybir.dt.int32
AF = mybir.ActivationFunctionType
OP = mybir.AluOpType
AX = mybir.AxisListType

# matmul N-chunks, each within one 512-float PSUM bank
N_CHUNKS = [(0, 512), (512, 1024), (1024, 1536), (1536, 1600)]


def _bcast_ap(handle, npart, inner_ap):
    """Manual AP over a DRAM/SBUF tensor replicated across npart partitions."""
    return bass.AP(tensor=handle.tensor, offset=handle.offset, ap=[[0, npart]] + inner_ap)


def build_kernel():
    nc = bacc.Bacc()

    logits_h = nc.declare_dram_parameter("logits", [Q, C], F32, isOutput=False)
    qbox_h = nc.declare_dram_parameter("qbox", [Q, 4], F32, isOutput=False)
    tbox_h = nc.declare_dram_parameter("tbox", [T, 4], F32, isOutput=False)
    tid_h = nc.declare_dram_parameter("tid", [T], I32, isOutput=False)
    out_h = nc.declare_dram_parameter("out", [Q, T], F32, isOutput=True)

    with ExitStack() as ctx:
        tc = ctx.enter_context(tile.TileContext(nc))
        consts = ctx.enter_context(tc.tile_pool(name="consts", bufs=1))
        rows = ctx.enter_context(tc.tile_pool(name="rows", bufs=1))
        qcols = ctx.enter_context(tc.tile_pool(name="qcols", bufs=1))
        ltile = ctx.enter_context(tc.tile_pool(name="ltile", bufs=2))
        longp = ctx.enter_context(tc.tile_pool(name="longp", bufs=5))
        tmp = ctx.enter_context(tc.tile_pool(name="tmp", bufs=9))
        ostage = ctx.enter_context(tc.tile_pool(name="ostage", bufs=2))
        gpsum = ctx.enter_context(tc.tile_pool(name="gpsum", bufs=1, space="PSUM"))
        tpsum = ctx.enter_context(tc.tile_pool(name="tpsum", bufs=2, space="PSUM"))

        # ---------------- constants ----------------
        ident = consts.tile([P, P], F32, tag="ident")
        make_identity(nc, ident)

        # class-index column [92,1] as f32 (for one-hot build)
        ic_i = consts.tile([C, 1], I32, tag="ic_i")
        nc.gpsimd.iota(ic_i, pattern=[[0, 1]], base=0, channel_multiplier=1)
        ic_f = consts.tile([C, 1], F32, tag="ic_f")
        nc.vector.tensor_copy(ic_f, ic_i)

        # one-hot [C, T]: onehot[c, t] = (tid[t] == c)
        tid_f = rows.tile([C, T], F32, tag="tid_f")
        tid_i = rows.tile([C, T], I32, tag="tid_i")
        nc.sync.dma_start(out=tid_i[:, :], in_=_bcast_ap(tid_h[:], C, [[1, T]]))
        nc.vector.tensor_copy(tid_f[:, :], tid_i[:, :])
        onehot = rows.tile([C, T], F32, tag="onehot")
        nc.vector.tensor_scalar(
            out=onehot, in0=tid_f, scalar1=ic_f[:, 0:1], scalar2=None, op0=OP.is_equal
        )

        # ---------------- target rows (broadcast to 128 partitions) -------------
        traw = rows.tile([P, T, 4], F32, tag="traw")
        nc.sync.dma_start(out=traw[:, :, :], in_=_bcast_ap(tbox_h[:, :], P, [[4, T], [1, 4]]))

        tx1b = traw[:, :, 0]
        ty1b = traw[:, :, 1]
        tx2b = traw[:, :, 2]
        ty2b = traw[:, :, 3]

        Rw = rows.tile([P, T], F32, tag="Rw")
        Rh = rows.tile([P, T], F32, tag="Rh")
        Ra4 = rows.tile([P, T], F32, tag="Ra4")
        Rcx = rows.tile([P, T], F32, tag="Rcx")
        Rcy = rows.tile([P, T], F32, tag="Rcy")
        Rat = rows.tile([P, T], F32, tag="Rat")

        nc.vector.tensor_tensor(out=Rw, in0=tx2b, in1=tx1b, op=OP.subtract)
        nc.vector.tensor_tensor(out=Rh, in0=ty2b, in1=ty1b, op=OP.subtract)
        # Ra4 = 4*tw*th
        nc.vector.scalar_tensor_tensor(
            out=Ra4, in0=Rw, scalar=4.0, in1=Rh, op0=OP.mult, op1=OP.mult
        )
        nc.vector.tensor_tensor(out=Rcx, in0=tx1b, in1=tx2b, op=OP.add)
        nc.vector.tensor_tensor(out=Rcy, in0=ty1b, in1=ty2b, op=OP.add)
        # Rat = arctan(tw / (th + EPS))   (unscaled; 2/pi folded into Square later)
        # ScalarE Arctan LUT only covers [-pi/2, pi/2]; range-reduce via
        # atan(r) = pi/2 - atan(1/r) for r > 1.
        def emit_atan(dst, wt, ht, mkt):
            t1 = mkt()
            nc.vector.tensor_scalar(out=t1, in0=ht, scalar1=EPS, scalar2=None, op0=OP.add)
            t2 = mkt()
            nc.vector.reciprocal_approx_fast(out=t2, in_=t1)
            r = mkt()
            nc.vector.tensor_tensor(out=r, in0=wt, in1=t2, op=OP.mult)
            ri = mkt()
            nc.vector.reciprocal_approx_fast(out=ri, in_=r)
            rc = mkt()
            nc.vector.tensor_tensor(out=rc, in0=r, in1=ri, op=OP.min)
            atc = mkt()
            nc.scalar.activation(out=atc, in_=rc, func=AF.Arctan)
            m = mkt()
            nc.vector.tensor_scalar(out=m, in0=r, scalar1=1.0, scalar2=None, op0=OP.is_gt)
            t3 = mkt()
            nc.vector.tensor_scalar(
                out=t3, in0=atc, scalar1=-2.0, scalar2=math.pi / 2.0, op0=OP.mult, op1=OP.add
            )
            mt = mkt()
            nc.vector.tensor_tensor(out=mt, in0=m, in1=t3, op=OP.mult)
            nc.vector.tensor_tensor(out=dst, in0=atc, in1=mt, op=OP.add)

        emit_atan(Rat, Rw, Rh, lambda: tmp.tile([P, T], F32, tag="tmp", name="att"))

        # ---------------- per-query columns ----------------
        qb = qcols.tile([P, NQT, 4], F32, tag="qb")
        nc.vector.memset(qb, 1.0)
        nc.vector.memset(qb[:, :, 0:2], 0.25)  # padding lanes: valid 0.75x0.75 box
        nfull = Q // P  # 7 full tiles
        nc.sync.dma_start(
            out=qb[:, 0:nfull, :],
            in_=bass.AP(
                tensor=qbox_h[:, :].tensor,
                offset=qbox_h[:, :].offset,
                ap=[[4, P], [P * 4, nfull], [1, 4]],
            ),
        )
        qrem = Q - nfull * P  # 4
        nc.sync.dma_start(out=qb[0:qrem, nfull, :], in_=qbox_h[nfull * P : Q, :])

        qx1 = qb[:, :, 0]
        qy1 = qb[:, :, 1]
        qx2 = qb[:, :, 2]
        qy2 = qb[:, :, 3]

        def qtile(tag):
            t = qcols.tile([P, NQT], F32, tag=tag)
            return t

        qw8 = qtile("qw8")
        qh8 = qtile("qh8")
        nc.vector.tensor_tensor(out=qw8, in0=qx2, in1=qx1, op=OP.subtract)
        nc.vector.tensor_tensor(out=qh8, in0=qy2, in1=qy1, op=OP.subtract)
        nqx1_8 = qtile("nqx1")
        nqy1_8 = qtile("nqy1")
        nqx2_8 = qtile("nqx2")
        nqy2_8 = qtile("nqy2")
        for dst, src in ((nqx1_8, qx1), (nqy1_8, qy1), (nqx2_8, qx2), (nqy2_8, qy2)):
            nc.vector.tensor_scalar(
                out=dst, in0=src, scalar1=-1.0, scalar2=None, op0=OP.mult
            )
        # qa4e = 4*qw*qh + 4*EPS
        qa4e8 = qtile("qa4e")
        nc.vector.scalar_tensor_tensor(
            out=qa4e8, in0=qw8, scalar=4.0, in1=qh8, op0=OP.mult, op1=OP.mult
        )
        nc.vector.tensor_scalar(
            out=qa4e8, in0=qa4e8, scalar1=4.0 * EPS, scalar2=None, op0=OP.add
        )
        # nqcx = -(qx1+qx2), nqcy = -(qy1+qy2)
        nqcx8 = qtile("nqcx")
        nqcy8 = qtile("nqcy")
        nc.vector.scalar_tensor_tensor(
            out=nqcx8, in0=qx1, scalar=-1.0, in1=qx2, op0=OP.mult, op1=OP.subtract
        )
        nc.vector.scalar_tensor_tensor(
            out=nqcy8, in0=qy1, scalar=-1.0, in1=qy2, op0=OP.mult, op1=OP.subtract
        )
        # nqat = -(2/pi) * arctan(qw / (qh + EPS))
        qat = qtile("qat")
        _qtc = [0]

        def _mkq():
            _qtc[0] += 1
            return qcols.tile([P, NQT], F32, tag=f"qat_t{_qtc[0]}", name="qat_t")

        emit_atan(qat, qw8, qh8, _mkq)
        nqat8 = qtile("nqat")
        nc.vector.tensor_scalar(
            out=nqat8, in0=qat, scalar1=-2.0 / math.pi, scalar2=None, op0=OP.mult
        )

        # ---------------- softmax numerator transposes ----------------
        mneg8 = qcols.tile([P, NQT], F32, tag="mneg8")
        ssum8 = qcols.tile([P, NQT], F32, tag="ssum8")
        nc.vector.memset(ssum8, 1.0)
        eT = qcols.tile([C, NQT, P], F32, tag="eT")

        for k in range(NQT):
            pk = min(P, Q - k * P)
            L = ltile.tile([P, C], F32, tag="L")
            nc.sync.dma_start(out=L[0:pk, :], in_=logits_h[k * P : k * P + pk, :])
            nc.vector.tensor_reduce(
                out=mneg8[0:pk, k : k + 1],
                in_=L[0:pk, :],
                axis=AX.X,
                op=OP.max,
                negate=True,
            )
            e = ltile.tile([P, C], F32, tag="e")
            nc.scalar.activation(
                out=e[0:pk, :],
                in_=L[0:pk, :],
                func=AF.Exp,
                bias=mneg8[0:pk, k : k + 1],
                scale=1.0,
                accum_out=ssum8[0:pk, k : k + 1],
            )
            tp = tpsum.tile([C, P], F32, tag="tp")
            nc.tensor.transpose(tp[:, 0:pk], e[0:pk, :], ident[0:pk, 0:pk])
            nc.scalar.copy(out=eT[:, k, 0:pk], in_=tp[:, 0:pk])

        # nr = -1/sum(exp)
        nr8 = qcols.tile([P, NQT], F32, tag="nr8")
        nc.vector.reciprocal(out=nr8, in_=ssum8)
        nc.vector.tensor_scalar(
            out=nr8, in0=nr8, scalar1=-1.0, scalar2=None, op0=OP.mult
        )

        # ---------------- main loop over query tiles ----------------
        for k in range(NQT):
            pk = min(P, Q - k * P)
            sl = slice(k, k + 1)

            # class gather: G[q, t] = exp(logit[q, tid[t]] - max_q)  (PE)
            g = gpsum.tile([P, T], F32, tag="g")
            for n0, n1 in N_CHUNKS:
                nc.tensor.matmul(
                    g[0:pk, n0:n1],
                    lhsT=eT[:, k, 0:pk],
                    rhs=onehot[:, n0:n1],
                    start=True,
                    stop=True,
                )

            def tt(tag, a, b, op, pool=tmp):
                o = pool.tile([P, T], F32, tag=tag)
                nc.vector.tensor_tensor(out=o[0:pk, :], in0=a, in1=b, op=op)
                return o

            def act(tag, in_, func, bias=0.0, scale=1.0, pool=tmp):
                o = pool.tile([P, T], F32, tag=tag)
                nc.scalar.activation(
                    out=o[0:pk, :], in_=in_, func=func, bias=bias, scale=scale
                )
                return o

            # |t - q| for the four coords (ACT)
            adx1 = act("tmp", tx1b[0:pk, :], AF.Abs, bias=nqx1_8[0:pk, sl])
            adx2 = act("tmp", tx2b[0:pk, :], AF.Abs, bias=nqx2_8[0:pk, sl])
            uX = tt("longp", adx1[0:pk, :], adx2[0:pk, :], OP.add, pool=longp)
            ady1 = act("tmp", ty1b[0:pk, :], AF.Abs, bias=nqy1_8[0:pk, sl])
            ady2 = act("tmp", ty2b[0:pk, :], AF.Abs, bias=nqy2_8[0:pk, sl])
            uY = tt("longp", ady1[0:pk, :], ady2[0:pk, :], OP.add, pool=longp)

            # intersection (scaled x4): relu(Sx-uX)*relu(Sy-uY)
            sxw = tt("tmp", Rw[0:pk, :], uX[0:pk, :], OP.subtract)
            px = act("tmp", sxw[0:pk, :], AF.Relu, bias=qw8[0:pk, sl])
            syw = tt("tmp", Rh[0:pk, :], uY[0:pk, :], OP.subtract)
            py = act("tmp", syw[0:pk, :], AF.Relu, bias=qh8[0:pk, sl])
            inter4 = tt("tmp", px[0:pk, :], py[0:pk, :], OP.mult)

            # -(4*union + 4eps) = inter4 - qa4e - Ra4
            nun = tmp.tile([P, T], F32, tag="tmp")
            nc.vector.scalar_tensor_tensor(
                out=nun[0:pk, :],
                in0=inter4[0:pk, :],
                scalar=qa4e8[0:pk, sl],
                in1=Ra4[0:pk, :],
                op0=OP.subtract,
                op1=OP.subtract,
            )
            rnu = tmp.tile([P, T], F32, tag="tmp")
            nc.vector.reciprocal_approx_fast(out=rnu[0:pk, :], in_=nun[0:pk, :])
            niou = longp.tile([P, T], F32, tag="longp")  # = -iou
            nc.vector.tensor_tensor(
                out=niou[0:pk, :], in0=inter4[0:pk, :], in1=rnu[0:pk, :], op=OP.mult
            )

            # convex diag (scaled x4): (Sx+uX)^2 + (Sy+uY)^2 + 4eps
            cwx = tt("tmp", Rw[0:pk, :], uX[0:pk, :], OP.add)
            sqcw = act("tmp", cwx[0:pk, :], AF.Square, bias=qw8[0:pk, sl])
            cwy = tt("tmp", Rh[0:pk, :], uY[0:pk, :], OP.add)
            sqch = act("tmp", cwy[0:pk, :], AF.Square, bias=qh8[0:pk, sl])
            diag = tmp.tile([P, T], F32, tag="tmp")
            nc.vector.scalar_tensor_tensor(
                out=diag[0:pk, :],
                in0=sqcw[0:pk, :],
                scalar=4.0 * EPS,
                in1=sqch[0:pk, :],
                op0=OP.add,
                op1=OP.add,
            )
            rd = tmp.tile([P, T], F32, tag="tmp")
            nc.vector.reciprocal_approx_fast(out=rd[0:pk, :], in_=diag[0:pk, :])

            # center distance (unscaled; /4 is absorbed by rd's x4)
            ex = act("tmp", Rcx[0:pk, :], AF.Square, bias=nqcx8[0:pk, sl])
            ey = act("tmp", Rcy[0:pk, :], AF.Square, bias=nqcy8[0:pk, sl])
            cd4 = tt("tmp", ex[0:pk, :], ey[0:pk, :], OP.add)
            pen = longp.tile([P, T], F32, tag="longp")
            nc.vector.tensor_tensor(
                out=pen[0:pk, :], in0=cd4[0:pk, :], in1=rd[0:pk, :], op=OP.mult
            )

            # v = ((2/pi)(atan_t - atan_q))^2 ; alpha*v = v^2/(1+eps-iou+v)
            v = act(
                "tmp", Rat[0:pk, :], AF.Square, bias=nqat8[0:pk, sl], scale=2.0 / math.pi
            )
            aden = tmp.tile([P, T], F32, tag="tmp")
            nc.vector.scalar_tensor_tensor(
                out=aden[0:pk, :],
                in0=niou[0:pk, :],
                scalar=1.0 + EPS,
                in1=v[0:pk, :],
                op0=OP.add,
                op1=OP.add,
            )
            ra = tmp.tile([P, T], F32, tag="tmp")
            nc.vector.reciprocal_approx_fast(out=ra[0:pk, :], in_=aden[0:pk, :])
            vsq = act("tmp", v[0:pk, :], AF.Square)
            av = tt("tmp", vsq[0:pk, :], ra[0:pk, :], OP.mult)

            # cost = L1 + (-iou) + pen + av + (-prob)
            L1 = tt("tmp", uX[0:pk, :], uY[0:pk, :], OP.add)
            s2 = tt("tmp", L1[0:pk, :], niou[0:pk, :], OP.add)
            s3 = tt("tmp", pen[0:pk, :], av[0:pk, :], OP.add)
            s4 = tt("tmp", s2[0:pk, :], s3[0:pk, :], OP.add)

            ost = ostage.tile([P, T], F32, tag="ostage")
            nc.vector.scalar_tensor_tensor(
                out=ost[0:pk, :],
                in0=g[0:pk, :],
                scalar=nr8[0:pk, sl],
                in1=s4[0:pk, :],
                op0=OP.mult,
                op1=OP.add,
            )
            nc.sync.dma_start(out=out_h[k * P : k * P + pk, :], in_=ost[0:pk, :])

    nc.compile()
    return nc


_NC_CACHE = None


def _get_nc():
    global _NC_CACHE
    if _NC_CACHE is None:
        _NC_CACHE = build_kernel()
    return _NC_CACHE


def kernel(pred_logits, pred_bbox, tgt_ids, tgt_bbox, **_unused):
    pred_logits = np.ascontiguousarray(np.asarray(pred_logits, dtype=np.float32))
    pred_bbox = np.ascontiguousarray(np.asarray(pred_bbox, dtype=np.float32))
    tgt_bbox = np.ascontiguousarray(np.asarray(tgt_bbox, dtype=np.float32))
    tid = np.ascontiguousarray(np.asarray(tgt_ids).astype(np.int32))

    nc = _get_nc()
    in_maps = [
        {
            "logits": pred_logits[i],
            "qbox": pred_bbox[i],
            "tbox": tgt_bbox,
            "tid": tid,
        }
        for i in range(B)
    ]
    res = run_bass_kernel_spmd(nc, in_maps, list(range(B)))
    out = np.stack([res.results[i]["out"] for i in range(B)], axis=0)
    return out.astype(np.float32)


if __name__ == "__main__":
    nc = build_kernel()
    print("built OK")


# revision 11
# speedup vs baseline: 1.4006x; 1.4006x over previous
"""Trainium2 Bass kernel: DETR-style bipartite matching cost matrix.

cost[b, q, t] = L1(pred_box, tgt_box) - softmax(logits)[q, tgt_id[t]] - CIoU(pred_box, tgt_box)

Sharding: data-parallel over batch. Core i computes the full [Q, T] cost slab
for batch i; tgt tensors are replicated. Host stacks the 8 slabs.
"""

import math
from contextlib import ExitStack

import numpy as np

import concourse.bass as bass
import concourse.bacc as bacc
import concourse.mybir as mybir
import concourse.tile as tile
from concourse.bass_utils import run_bass_kernel_spmd
from concourse.masks import make_identity

B, Q, C, T = 8, 900, 92, 1600
REPEAT = 1  # timing builds repeat the main loop
EPS = 1e-6
P = 128
NQT = (Q + P - 1) // P  # 8 query tiles; last is ragged (4 rows)
F32 = mybir.dt.float32
I32 = mybir.dt.int32
AF = mybir.ActivationFunctionType
OP = mybir.AluOpType
AX = mybir.AxisListType

# matmul N-chunks, each within one 512-float PSUM bank
N_CHUNKS = [(0, 512), (512, 1024), (1024, 1536), (1536, 1600)]


def _bcast_ap(handle, npart, inner_ap):
    """Manual AP over a DRAM/SBUF tensor replicated across npart partitions."""
    return bass.AP(tensor=handle.tensor, offset=handle.offset, ap=[[0, npart]] + inner_ap)


def build_kernel():
    nc = bacc.Bacc()

    logits_h = nc.declare_dram_parameter("logits", [Q, C], F32, isOutput=False)
    qbox_h = nc.declare_dram_parameter("qbox", [Q, 4], F32, isOutput=False)
    tbox_h = nc.declare_dram_parameter("tbox", [T, 4], F32, isOutput=False)
    tid_h = nc.declare_dram_parameter("tid", [T], I32, isOutput=False)
    out_h = nc.declare_dram_parameter("out", [Q, T], F32, isOutput=True)

    with ExitStack() as ctx:
        tc = ctx.enter_context(tile.TileContext(nc))
        consts = ctx.enter_context(tc.tile_pool(name="consts", bufs=1))
        rows = ctx.enter_context(tc.tile_pool(name="rows", bufs=1))
        qcols = ctx.enter_context(tc.tile_pool(name="qcols", bufs=1))
        ltile = ctx.enter_context(tc.tile_pool(name="ltile", bufs=2))
        longp = ctx.enter_context(tc.tile_pool(name="longp", bufs=5))
        tmp = ctx.enter_context(tc.tile_pool(name="tmp", bufs=9))
        ostage = ctx.enter_context(tc.tile_pool(name="ostage", bufs=2))
        gpsum = ctx.enter_context(tc.tile_pool(name="gpsum", bufs=1, space="PSUM"))
        tpsum = ctx.enter_context(tc.tile_pool(name="tpsum", bufs=2, space="PSUM"))

        # ---------------- constants ----------------
        ident = consts.tile([P, P], F32, tag="ident")
        make_identity(nc, ident)

        # class-index column [92,1] as f32 (for one-hot build)
        ic_i = consts.tile([C, 1], I32, tag="ic_i")
        nc.gpsimd.iota(ic_i, pattern=[[0, 1]], base=0, channel_multiplier=1)
        ic_f = consts.tile([C, 1], F32, tag="ic_f")
        nc.vector.tensor_copy(ic_f, ic_i)

        # one-hot [C, T]: onehot[c, t] = (tid[t] == c)
        tid_f = rows.tile([C, T], F32, tag="tid_f")
        tid_i = rows.tile([C, T], I32, tag="tid_i")
        nc.sync.dma_start(out=tid_i[:, :], in_=_bcast_ap(tid_h[:], C, [[1, T]]))
        nc.vector.tensor_copy(tid_f[:, :], tid_i[:, :])
        onehot = rows.tile([C, T], F32, tag="onehot")
        nc.vector.tensor_scalar(
            out=onehot, in0=tid_f, scalar1=ic_f[:, 0:1], scalar2=None, op0=OP.is_equal
        )

        # ---------------- target rows (broadcast to 128 partitions) -------------
        traw = rows.tile([P, T, 4], F32, tag="traw")
        nc.sync.dma_start(out=traw[:, :, :], in_=_bcast_ap(tbox_h[:, :], P, [[4, T], [1, 4]]))

        tx1b = traw[:, :, 0]
        ty1b = traw[:, :, 1]
        tx2b = traw[:, :, 2]
        ty2b = traw[:, :, 3]

        Rw = rows.tile([P, T], F32, tag="Rw")
        Rh = rows.tile([P, T], F32, tag="Rh")
        Ra4 = rows.tile([P, T], F32, tag="Ra4")
        Rcx = rows.tile([P, T], F32, tag="Rcx")
        Rcy = rows.tile([P, T], F32, tag="Rcy")
        Rat = rows.tile([P, T], F32, tag="Rat")

        nc.vector.tensor_tensor(out=Rw, in0=tx2b, in1=tx1b, op=OP.subtract)
        nc.vector.tensor_tensor(out=Rh, in0=ty2b, in1=ty1b, op=OP.subtract)
        # Ra4 = 4*tw*th
        nc.vector.scalar_tensor_tensor(
            out=Ra4, in0=Rw, scalar=4.0, in1=Rh, op0=OP.mult, op1=OP.mult
        )
        nc.vector.tensor_tensor(out=Rcx, in0=tx1b, in1=tx2b, op=OP.add)
        nc.vector.tensor_tensor(out=Rcy, in0=ty1b, in1=ty2b, op=OP.add)
        # Rat = arctan(tw / (th + EPS))   (unscaled; 2/pi folded into Square later)
        # ScalarE Arctan LUT only covers [-pi/2, pi/2]; range-reduce via
        # atan(r) = pi/2 - atan(1/r) for r > 1.
        def emit_atan(dst, wt, ht, mkt):
            t1 = mkt()
            nc.vector.tensor_scalar(out=t1, in0=ht, scalar1=EPS, scalar2=None, op0=OP.add)
            t2 = mkt()
            nc.vector.reciprocal_approx_fast(out=t2, in_=t1)
            r = mkt()
            nc.vector.tensor_tensor(out=r, in0=wt, in1=t2, op=OP.mult)
            ri = mkt()
            nc.vector.reciprocal_approx_fast(out=ri, in_=r)
            rc = mkt()
            nc.vector.tensor_tensor(out=rc, in0=r, in1=ri, op=OP.min)
            atc = mkt()
            nc.scalar.activation(out=atc, in_=rc, func=AF.Arctan)
            m = mkt()
            nc.vector.tensor_scalar(out=m, in0=r, scalar1=1.0, scalar2=None, op0=OP.is_gt)
            t3 = mkt()
            nc.vector.tensor_scalar(
                out=t3, in0=atc, scalar1=-2.0, scalar2=math.pi / 2.0, op0=OP.mult, op1=OP.add
            )
            mt = mkt()
            nc.vector.tensor_tensor(out=mt, in0=m, in1=t3, op=OP.mult)
            nc.vector.tensor_tensor(out=dst, in0=atc, in1=mt, op=OP.add)

        emit_atan(Rat, Rw, Rh, lambda: tmp.tile([P, T], F32, tag="tmp", name="att"))

        # ---------------- per-query columns ----------------
        qb = qcols.tile([P, NQT, 4], F32, tag="qb")
        nc.vector.memset(qb, 1.0)
        nc.vector.memset(qb[:, :, 0:2], 0.25)  # padding lanes: valid 0.75x0.75 box
        nfull = Q // P  # 7 full tiles
        nc.sync.dma_start(
            out=qb[:, 0:nfull, :],
            in_=bass.AP(
                tensor=qbox_h[:, :].tensor,
                offset=qbox_h[:, :].offset,
                ap=[[4, P], [P * 4, nfull], [1, 4]],
            ),
        )
        qrem = Q - nfull * P  # 4
        nc.sync.dma_start(out=qb[0:qrem, nfull, :], in_=qbox_h[nfull * P : Q, :])

        qx1 = qb[:, :, 0]
        qy1 = qb[:, :, 1]
        qx2 = qb[:, :, 2]
        qy2 = qb[:, :, 3]

        def qtile(tag):
            t = qcols.tile([P, NQT], F32, tag=tag)
            return t

        qw8 = qtile("qw8")
        qh8 = qtile("qh8")
        nc.vector.tensor_tensor(out=qw8, in0=qx2, in1=qx1, op=OP.subtract)
        nc.vector.tensor_tensor(out=qh8, in0=qy2, in1=qy1, op=OP.subtract)
        nqx1_8 = qtile("nqx1")
        nqy1_8 = qtile("nqy1")
        nqx2_8 = qtile("nqx2")
        nqy2_8 = qtile("nqy2")
        for dst, src in ((nqx1_8, qx1), (nqy1_8, qy1), (nqx2_8, qx2), (nqy2_8, qy2)):
            nc.vector.tensor_scalar(
                out=dst, in0=src, scalar1=-1.0, scalar2=None, op0=OP.mult
            )
        # qa4e = 4*qw*qh + 4*EPS
        qa4e8 = qtile("qa4e")
        nc.vector.scalar_tensor_tensor(
            out=qa4e8, in0=qw8, scalar=4.0, in1=qh8, op0=OP.mult, op1=OP.mult
        )
        nc.vector.tensor_scalar(
            out=qa4e8, in0=qa4e8, scalar1=4.0 * EPS, scalar2=None, op0=OP.add
        )
        # nqcx = -(qx1+qx2), nqcy = -(qy1+qy2)
        nqcx8 = qtile("nqcx")
        nqcy8 = qtile("nqcy")
        nc.vector.scalar_tensor_tensor(
            out=nqcx8, in0=qx1, scalar=-1.0, in1=qx2, op0=OP.mult, op1=OP.subtract
        )
        nc.vector.scalar_tensor_tensor(
            out=nqcy8, in0=qy1, scalar=-1.0, in1=qy2, op0=OP.mult, op1=OP.subtract
        )
        # nqat = -(2/pi) * arctan(qw / (qh + EPS))
        qat = qtile("qat")
        _qtc = [0]

        def _mkq():
            _qtc[0] += 1
            return qcols.tile([P, NQT], F32, tag=f"qat_t{_qtc[0]}", name="qat_t")

        emit_atan(qat, qw8, qh8, _mkq)
        nqat8 = qtile("nqat")
        nc.vector.tensor_scalar(
            out=nqat8, in0=qat, scalar1=-2.0 / math.pi, scalar2=None, op0=OP.mult
        )

        # ---------------- softmax numerator transposes ----------------
        mneg8 = qcols.tile([P, NQT], F32, tag="mneg8")
        ssum8 = qcols.tile([P, NQT], F32, tag="ssum8")
        nc.vector.memset(ssum8, 1.0)
        eT = qcols.tile([C, NQT, P], F32, tag="eT")

        for k in range(NQT):
            pk = min(P, Q - k * P)
            L = ltile.tile([P, C], F32, tag="L")
            nc.sync.dma_start(out=L[0:pk, :], in_=logits_h[k * P : k * P + pk, :])
            nc.vector.tensor_reduce(
                out=mneg8[0:pk, k : k + 1],
                in_=L[0:pk, :],
                axis=AX.X,
                op=OP.max,
                negate=True,
            )
            e = ltile.tile([P, C], F32, tag="e")
            nc.scalar.activation(
                out=e[0:pk, :],
                in_=L[0:pk, :],
                func=AF.Exp,
                bias=mneg8[0:pk, k : k + 1],
                scale=1.0,
                accum_out=ssum8[0:pk, k : k + 1],
            )
            tp = tpsum.tile([C, P], F32, tag="tp")
            nc.tensor.transpose(tp[:, 0:pk], e[0:pk, :], ident[0:pk, 0:pk])
            nc.scalar.copy(out=eT[:, k, 0:pk], in_=tp[:, 0:pk])

        # nr = -1/sum(exp)
        nr8 = qcols.tile([P, NQT], F32, tag="nr8")
        nc.vector.reciprocal(out=nr8, in_=ssum8)
        nc.vector.tensor_scalar(
            out=nr8, in0=nr8, scalar1=-1.0, scalar2=None, op0=OP.mult
        )

        # ---------------- main loop over query tiles ----------------
        for k in [kk for _rep in range(REPEAT) for kk in range(NQT)]:
            pk = min(P, Q - k * P)
            sl = slice(k, k + 1)

            # class gather: G[q, t] = exp(logit[q, tid[t]] - max_q)  (PE)
            g = gpsum.tile([P, T], F32, tag="g")
            for n0, n1 in N_CHUNKS:
                nc.tensor.matmul(
                    g[0:pk, n0:n1],
                    lhsT=eT[:, k, 0:pk],
                    rhs=onehot[:, n0:n1],
                    start=True,
                    stop=True,
                )

            def tt(tag, a, b, op, pool=tmp):
                o = pool.tile([P, T], F32, tag=tag)
                nc.vector.tensor_tensor(out=o[0:pk, :], in0=a, in1=b, op=op)
                return o

            def act(tag, in_, func, bias=0.0, scale=1.0, pool=tmp):
                o = pool.tile([P, T], F32, tag=tag)
                nc.scalar.activation(
                    out=o[0:pk, :], in_=in_, func=func, bias=bias, scale=scale
                )
                return o

            # |t - q| for the four coords (ACT)
            adx1 = act("tmp", tx1b[0:pk, :], AF.Abs, bias=nqx1_8[0:pk, sl])
            adx2 = act("tmp", tx2b[0:pk, :], AF.Abs, bias=nqx2_8[0:pk, sl])
            uX = tt("longp", adx1[0:pk, :], adx2[0:pk, :], OP.add, pool=longp)
            ady1 = act("tmp", ty1b[0:pk, :], AF.Abs, bias=nqy1_8[0:pk, sl])
            ady2 = act("tmp", ty2b[0:pk, :], AF.Abs, bias=nqy2_8[0:pk, sl])
            uY = tt("longp", ady1[0:pk, :], ady2[0:pk, :], OP.add, pool=longp)

            # intersection (scaled x4): relu(Sx-uX)*relu(Sy-uY)
            sxw = tt("tmp", Rw[0:pk, :], uX[0:pk, :], OP.subtract)
            px = act("tmp", sxw[0:pk, :], AF.Relu, bias=qw8[0:pk, sl])
            syw = tt("tmp", Rh[0:pk, :], uY[0:pk, :], OP.subtract)
            py = act("tmp", syw[0:pk, :], AF.Relu, bias=qh8[0:pk, sl])
            inter4 = tt("tmp", px[0:pk, :], py[0:pk, :], OP.mult)

            # -(4*union + 4eps) = inter4 - qa4e - Ra4
            nun = tmp.tile([P, T], F32, tag="tmp")
            nc.vector.scalar_tensor_tensor(
                out=nun[0:pk, :],
                in0=inter4[0:pk, :],
                scalar=qa4e8[0:pk, sl],
                in1=Ra4[0:pk, :],
                op0=OP.subtract,
                op1=OP.subtract,
            )
            rnu = tmp.tile([P, T], F32, tag="tmp")
            nc.vector.reciprocal_approx_fast(out=rnu[0:pk, :], in_=nun[0:pk, :])
            niou = longp.tile([P, T], F32, tag="longp")  # = -iou
            nc.vector.tensor_tensor(
                out=niou[0:pk, :], in0=inter4[0:pk, :], in1=rnu[0:pk, :], op=OP.mult
            )

            # convex diag (scaled x4): (Sx+uX)^2 + (Sy+uY)^2 + 4eps
            cwx = tt("tmp", Rw[0:pk, :], uX[0:pk, :], OP.add)
            sqcw = act("tmp", cwx[0:pk, :], AF.Square, bias=qw8[0:pk, sl])
            cwy = tt("tmp", Rh[0:pk, :], uY[0:pk, :], OP.add)
            sqch = act("tmp", cwy[0:pk, :], AF.Square, bias=qh8[0:pk, sl])
            diag = tmp.tile([P, T], F32, tag="tmp")
            nc.vector.scalar_tensor_tensor(
                out=diag[0:pk, :],
                in0=sqcw[0:pk, :],
                scalar=4.0 * EPS,
                in1=sqch[0:pk, :],
                op0=OP.add,
                op1=OP.add,
            )
            rd = tmp.tile([P, T], F32, tag="tmp")
            nc.vector.reciprocal_approx_fast(out=rd[0:pk, :], in_=diag[0:pk, :])

            # center distance (unscaled; /4 is absorbed by rd's x4)
            ex = act("tmp", Rcx[0:pk, :], AF.Square, bias=nqcx8[0:pk, sl])
            ey = act("tmp", Rcy[0:pk, :], AF.Square, bias=nqcy8[0:pk, sl])
            cd4 = tt("tmp", ex[0:pk, :], ey[0:pk, :], OP.add)
            pen = longp.tile([P, T], F32, tag="longp")
            nc.vector.tensor_tensor(
                out=pen[0:pk, :], in0=cd4[0:pk, :], in1=rd[0:pk, :], op=OP.mult
            )

            # v = ((2/pi)(atan_t - atan_q))^2 ; alpha*v = v^2/(1+eps-iou+v)
            v = act(
                "tmp", Rat[0:pk, :], AF.Square, bias=nqat8[0:pk, sl], scale=2.0 / math.pi
            )
            aden = tmp.tile([P, T], F32, tag="tmp")
            nc.vector.scalar_tensor_tensor(
                out=aden[0:pk, :],
                in0=niou[0:pk, :],
                scalar=1.0 + EPS,
                in1=v[0:pk, :],
                op0=OP.add,
                op1=OP.add,
            )
            ra = tmp.tile([P, T], F32, tag="tmp")
            nc.vector.reciprocal_approx_fast(out=ra[0:pk, :], in_=aden[0:pk, :])
            vsq = act("tmp", v[0:pk, :], AF.Square)
            av = tt("tmp", vsq[0:pk, :], ra[0:pk, :], OP.mult)

            # cost = L1 + (-iou) + pen + av + (-prob)
            L1 = tt("tmp", uX[0:pk, :], uY[0:pk, :], OP.add)
            s2 = tt("tmp", L1[0:pk, :], niou[0:pk, :], OP.add)
            s3 = tt("tmp", pen[0:pk, :], av[0:pk, :], OP.add)
            s4 = tt("tmp", s2[0:pk, :], s3[0:pk, :], OP.add)

            ost = ostage.tile([P, T], F32, tag="ostage")
            nc.vector.scalar_tensor_tensor(
                out=ost[0:pk, :],
                in0=g[0:pk, :],
                scalar=nr8[0:pk, sl],
                in1=s4[0:pk, :],
                op0=OP.mult,
                op1=OP.add,
            )
            nc.sync.dma_start(out=out_h[k * P : k * P + pk, :], in_=ost[0:pk, :])

    nc.compile()
    return nc


_NC_CACHE = None


def _get_nc():
    global _NC_CACHE
    if _NC_CACHE is None:
        _NC_CACHE = build_kernel()
    return _NC_CACHE


def kernel(pred_logits, pred_bbox, tgt_ids, tgt_bbox, **_unused):
    pred_logits = np.ascontiguousarray(np.asarray(pred_logits, dtype=np.float32))
    pred_bbox = np.ascontiguousarray(np.asarray(pred_bbox, dtype=np.float32))
    tgt_bbox = np.ascontiguousarray(np.asarray(tgt_bbox, dtype=np.float32))
    tid = np.ascontiguousarray(np.asarray(tgt_ids).astype(np.int32))

    nc = _get_nc()
    in_maps = [
        {
            "logits": pred_logits[i],
            "qbox": pred_bbox[i],
            "tbox": tgt_bbox,
            "tid": tid,
        }
        for i in range(B)
    ]
    res = run_bass_kernel_spmd(nc, in_maps, list(range(B)))
    out = np.stack([res.results[i]["out"] for i in range(B)], axis=0)
    return out.astype(np.float32)


if __name__ == "__main__":
    nc = build_kernel()
    print("built OK")


# revision 12
# speedup vs baseline: 4.6211x; 3.2993x over previous
"""Trainium2 Bass kernel v2: fp16 interior + TensorE-accumulated final sums.

cost[b, q, t] = L1(pred_box, tgt_box) - softmax(logits)[q, tgt_id[t]] - CIoU(pred_box, tgt_box)

Per-core (batch-parallel) plan, queries on partitions, targets on free dim.
The final cost = (-prob) + L1 + (-iou) + pen + alpha*v is accumulated in PSUM:
the class term via an fp16 matmul expT_scaled @ onehot, the four per-pair
addends via fp16 identity matmuls. fp32 is kept on the reciprocal chains.
"""

import math
from contextlib import ExitStack

import numpy as np

import concourse.bass as bass
import concourse.bacc as bacc
import concourse.mybir as mybir
import concourse.tile as tile
from concourse.bass_utils import run_bass_kernel_spmd
from concourse.masks import make_identity

B, Q, C, T = 8, 900, 92, 1600
REPEAT = 1
EPS = 1e-6
P = 128
NQT = (Q + P - 1) // P  # 8 query tiles; last is ragged (4 rows)
F32 = mybir.dt.float32
F16 = mybir.dt.float16
I32 = mybir.dt.int32
AF = mybir.ActivationFunctionType
OP = mybir.AluOpType
AX = mybir.AxisListType

N_CHUNKS = [(0, 512), (512, 1024), (1024, 1536), (1536, 1600)]


def _bcast_ap(ap, npart, inner_ap):
    return bass.AP(tensor=ap.tensor, offset=ap.offset, ap=[[0, npart]] + inner_ap)


def build_kernel():
    nc = bacc.Bacc()

    logits_h = nc.declare_dram_parameter("logits", [Q, C], F32, isOutput=False)
    qbox_h = nc.declare_dram_parameter("qbox", [Q, 4], F32, isOutput=False)
    tbox_h = nc.declare_dram_parameter("tbox", [T, 4], F32, isOutput=False)
    tid_h = nc.declare_dram_parameter("tid", [T], I32, isOutput=False)
    out_h = nc.declare_dram_parameter("out", [Q, T], F32, isOutput=True)

    with ExitStack() as ctx:
        tc = ctx.enter_context(tile.TileContext(nc))
        consts = ctx.enter_context(tc.tile_pool(name="consts", bufs=1))
        rows = ctx.enter_context(tc.tile_pool(name="rows", bufs=1))
        qcols = ctx.enter_context(tc.tile_pool(name="qcols", bufs=1))

        # ---------------- constants ----------------
        ident_h = consts.tile([P, P], F16, tag="ident_h")
        make_identity(nc, ident_h)
        ic_i = consts.tile([C, 1], I32, tag="ic_i")
        nc.gpsimd.iota(ic_i, pattern=[[0, 1]], base=0, channel_multiplier=1)
        ic_f = consts.tile([C, 1], F32, tag="ic_f")
        nc.vector.tensor_copy(ic_f, ic_i)

        # persistent target rows
        traw = rows.tile([P, T, 4], F32, tag="traw")
        RwH = rows.tile([P, T], F16, tag="RwH")
        RhH = rows.tile([P, T], F16, tag="RhH")
        Ra4 = rows.tile([P, T], F32, tag="Ra4")
        Rcx = rows.tile([P, T], F32, tag="Rcx")
        Rcy = rows.tile([P, T], F32, tag="Rcy")
        Rat = rows.tile([P, T], F32, tag="Rat")
        onehot16 = rows.tile([C, T], F16, tag="onehot16")

        # tail tiles pre-allocated here so no new tags land in rows/qcols after
        # the scratch pool below is freed (Tile mis-syncs recycled regions)
        TQ, TC, TW = 4, 32, 50
        q0 = Q - TQ  # 896
        NFULL = NQT - 1
        trawt = rows.tile([P, TW, 4], F32, tag="trawt")
        Rw32t = rows.tile([P, TW], F32, tag="Rw32t")
        Rh32t = rows.tile([P, TW], F32, tag="Rh32t")
        RwHt = rows.tile([P, TW], F16, tag="RwHt")
        RhHt = rows.tile([P, TW], F16, tag="RhHt")
        Ra4t = rows.tile([P, TW], F32, tag="Ra4t")
        Rcxt = rows.tile([P, TW], F32, tag="Rcxt")
        Rcyt = rows.tile([P, TW], F32, tag="Rcyt")
        Ratt = rows.tile([P, TW], F32, tag="Ratt")
        tqb = qcols.tile([P, 4], F32, tag="tqb")

        def ttile(tag):
            return qcols.tile([P, 1], F32, tag=tag, name=tag)

        tqw = ttile("tqw")
        tqh = ttile("tqh")
        tnqx1 = ttile("tnqx1")
        tnqy1 = ttile("tnqy1")
        tnqx2 = ttile("tnqx2")
        tnqy2 = ttile("tnqy2")
        tqa4e = ttile("tqa4e")
        tnqcx = ttile("tnqcx")
        tnqcy = ttile("tnqcy")
        tqat = ttile("tqat")
        tnqat = ttile("tnqat")
        _tat_tiles = [ttile(f"tat{i}") for i in range(9)]

        # broadcast raw tbox [1600,4] to all partitions (doubling DMA)
        nc.sync.dma_start(
            out=traw[:, :, :], in_=_bcast_ap(tbox_h[:, :], P, [[4, T], [1, 4]])
        )

        tx1b = traw[:, :, 0]
        ty1b = traw[:, :, 1]
        tx2b = traw[:, :, 2]
        ty2b = traw[:, :, 3]

        # arctan with range reduction: atan(r) = pi/2 - atan(1/r) for r > 1
        def emit_atan(dst, wt, ht, mkt):
            t1 = mkt()
            nc.vector.tensor_scalar(
                out=t1, in0=ht, scalar1=EPS, scalar2=None, op0=OP.add
            )
            t2 = mkt()
            nc.vector.reciprocal_approx_fast(out=t2, in_=t1)
            r = mkt()
            nc.vector.tensor_tensor(out=r, in0=wt, in1=t2, op=OP.mult)
            ri = mkt()
            nc.vector.reciprocal_approx_fast(out=ri, in_=r)
            rc = mkt()
            nc.vector.tensor_tensor(out=rc, in0=r, in1=ri, op=OP.min)
            atc = mkt()
            nc.scalar.activation(out=atc, in_=rc, func=AF.Arctan)
            m = mkt()
            nc.vector.tensor_scalar(
                out=m, in0=r, scalar1=1.0, scalar2=None, op0=OP.is_gt
            )
            t3 = mkt()
            nc.vector.tensor_scalar(
                out=t3,
                in0=atc,
                scalar1=-2.0,
                scalar2=math.pi / 2.0,
                op0=OP.mult,
                op1=OP.add,
            )
            mt = mkt()
            nc.vector.tensor_tensor(out=mt, in0=m, in1=t3, op=OP.mult)
            nc.vector.tensor_tensor(out=dst, in0=atc, in1=mt, op=OP.add)

        # scratch pool: freed before the main loop pools open
        with tc.tile_pool(name="scratch", bufs=1) as scratch:
            # one-hot from tgt ids
            tid_i = scratch.tile([C, T], I32, tag="tid_i")
            nc.sync.dma_start(out=tid_i[:, :], in_=_bcast_ap(tid_h[:], C, [[1, T]]))
            tid_f = scratch.tile([C, T], F32, tag="tid_f")
            nc.vector.tensor_copy(tid_f[:, :], tid_i[:, :])
            oh32 = scratch.tile([C, T], F32, tag="oh32")
            nc.vector.tensor_scalar(
                out=oh32, in0=tid_f, scalar1=ic_f[:, 0:1], scalar2=None, op0=OP.is_equal
            )
            nc.vector.tensor_copy(onehot16[:, :], oh32[:, :])

            Rw = scratch.tile([P, T], F32, tag="Rw")
            Rh = scratch.tile([P, T], F32, tag="Rh")
            nc.vector.tensor_tensor(out=Rw, in0=tx2b, in1=tx1b, op=OP.subtract)
            nc.vector.tensor_tensor(out=Rh, in0=ty2b, in1=ty1b, op=OP.subtract)
            nc.vector.tensor_copy(RwH[:, :], Rw[:, :])
            nc.vector.tensor_copy(RhH[:, :], Rh[:, :])
            nc.vector.scalar_tensor_tensor(
                out=Ra4, in0=Rw, scalar=4.0, in1=Rh, op0=OP.mult, op1=OP.mult
            )
            nc.vector.tensor_tensor(out=Rcx, in0=tx1b, in1=tx2b, op=OP.add)
            nc.vector.tensor_tensor(out=Rcy, in0=ty1b, in1=ty2b, op=OP.add)

            _atc = [0]

            def _mka():
                _atc[0] += 1
                return scratch.tile([P, T], F32, tag="att", name="att", bufs=5)

            emit_atan(Rat, Rw, Rh, _mka)

            # ------------- per-query columns (inside scratch epoch is fine; they
            # live in qcols which persists) -------------
            qb = qcols.tile([P, NQT, 4], F32, tag="qb")
            nc.vector.memset(qb, 1.0)
            nc.vector.memset(qb[:, :, 0:2], 0.25)
            nfull = Q // P
            nc.sync.dma_start(
                out=qb[:, 0:nfull, :],
                in_=bass.AP(
                    tensor=qbox_h[:, :].tensor,
                    offset=qbox_h[:, :].offset,
                    ap=[[4, P], [P * 4, nfull], [1, 4]],
                ),
            )
            nc.sync.dma_start(out=qb[0 : Q - nfull * P, nfull, :], in_=qbox_h[nfull * P : Q, :])

            qx1 = qb[:, :, 0]
            qy1 = qb[:, :, 1]
            qx2 = qb[:, :, 2]
            qy2 = qb[:, :, 3]

            def qtile(tag):
                return qcols.tile([P, NQT], F32, tag=tag, name=tag)

            qw8 = qtile("qw8")
            qh8 = qtile("qh8")
            nc.vector.tensor_tensor(out=qw8, in0=qx2, in1=qx1, op=OP.subtract)
            nc.vector.tensor_tensor(out=qh8, in0=qy2, in1=qy1, op=OP.subtract)
            nqx1_8 = qtile("nqx1")
            nqy1_8 = qtile("nqy1")
            nqx2_8 = qtile("nqx2")
            nqy2_8 = qtile("nqy2")
            for dst, src in (
                (nqx1_8, qx1),
                (nqy1_8, qy1),
                (nqx2_8, qx2),
                (nqy2_8, qy2),
            ):
                nc.vector.tensor_scalar(
                    out=dst, in0=src, scalar1=-1.0, scalar2=None, op0=OP.mult
                )
            qa4e8 = qtile("qa4e")
            nc.vector.scalar_tensor_tensor(
                out=qa4e8, in0=qw8, scalar=4.0, in1=qh8, op0=OP.mult, op1=OP.mult
            )
            nc.vector.tensor_scalar(
                out=qa4e8, in0=qa4e8, scalar1=4.0 * EPS, scalar2=None, op0=OP.add
            )
            nqcx8 = qtile("nqcx")
            nqcy8 = qtile("nqcy")
            nc.vector.scalar_tensor_tensor(
                out=nqcx8, in0=qx1, scalar=-1.0, in1=qx2, op0=OP.mult, op1=OP.subtract
            )
            nc.vector.scalar_tensor_tensor(
                out=nqcy8, in0=qy1, scalar=-1.0, in1=qy2, op0=OP.mult, op1=OP.subtract
            )
            qat = qtile("qat")
            _qtc = [0]

            def _mkq():
                _qtc[0] += 1
                return qcols.tile([P, NQT], F32, tag=f"qat_t{_qtc[0]}", name="qat_t")

            emit_atan(qat, qw8, qh8, _mkq)
            nqat8 = qtile("nqat")
            nc.vector.tensor_scalar(
                out=nqat8, in0=qat, scalar1=-2.0 / math.pi, scalar2=None, op0=OP.mult
            )

            # ------------- softmax (phase A): exp + row sums -------------
            mneg8 = qcols.tile([P, NQT], F32, tag="mneg8")
            ssum8 = qcols.tile([P, NQT], F32, tag="ssum8")
            nc.vector.memset(ssum8, 1.0)
            e_all = qcols.tile([P, NQT, C], F32, tag="e_all")

            for k in range(NQT):
                pk = min(P, Q - k * P)
                L = scratch.tile([P, C], F32, tag="L", name="L", bufs=3)
                nc.sync.dma_start(
                    out=L[0:pk, :], in_=logits_h[k * P : k * P + pk, :]
                )
                nc.vector.tensor_reduce(
                    out=mneg8[0:pk, k : k + 1],
                    in_=L[0:pk, :],
                    axis=AX.X,
                    op=OP.max,
                    negate=True,
                )
                nc.scalar.activation(
                    out=e_all[0:pk, k, :],
                    in_=L[0:pk, :],
                    func=AF.Exp,
                    bias=mneg8[0:pk, k : k + 1],
                    scale=1.0,
                    accum_out=ssum8[0:pk, k : k + 1],
                )

            # nr = -1/sum(exp)
            nr8 = qcols.tile([P, NQT], F32, tag="nr8")
            nc.vector.reciprocal(out=nr8, in_=ssum8)
            nc.vector.tensor_scalar(
                out=nr8, in0=nr8, scalar1=-1.0, scalar2=None, op0=OP.mult
            )

        # ------------- softmax (phase B): scale by -1/sum, transpose (fp16) ----
        eT = qcols.tile([C, NQT, P], F16, tag="eT")
        with tc.tile_pool(name="tposep", bufs=2, space="PSUM") as tpsum, tc.tile_pool(
            name="es16", bufs=2
        ) as es16:
            for k in range(NQT):
                pk = min(P, Q - k * P)
                es = es16.tile([P, C], F16, tag="es", name="es")
                nc.vector.tensor_scalar(
                    out=es[0:pk, :],
                    in0=e_all[0:pk, k, :],
                    scalar1=nr8[0:pk, k : k + 1],
                    scalar2=None,
                    op0=OP.mult,
                )
                tp = tpsum.tile([C, P], F16, tag="tp", name="tp")
                nc.tensor.transpose(tp[:, 0:pk], es[0:pk, :], ident_h[0:pk, 0:pk])
                nc.scalar.copy(out=eT[:, k, 0:pk], in_=tp[:, 0:pk])

        # ---------------- main loop pools ----------------
        long16 = ctx.enter_context(tc.tile_pool(name="long16", bufs=5))
        add16 = ctx.enter_context(tc.tile_pool(name="add16", bufs=6))
        tmp16 = ctx.enter_context(tc.tile_pool(name="tmp16", bufs=10))
        tmp32 = ctx.enter_context(tc.tile_pool(name="tmp32", bufs=6))
        ostage = ctx.enter_context(tc.tile_pool(name="ostage", bufs=2))
        gpsum = ctx.enter_context(tc.tile_pool(name="gpsum", bufs=2, space="PSUM"))

        def emit_dag(pk, fd, g, chunks, cols, trows, class_starts):
            """Emit the per-pair cost DAG into PSUM tile `g` ([pk, fd] region).

            cols: per-query [pk,1] APs; trows: target-row APs at [pk, fd].
            If class_starts, the class matmuls already started the PSUM group.
            """
            first = [not class_starts]

            def accum(x, stop):
                st = first[0]
                first[0] = False
                for n0, n1 in chunks:
                    nc.tensor.matmul(
                        g[0:pk, n0:n1],
                        lhsT=ident_h[0:pk, 0:pk],
                        rhs=x[0:pk, n0:n1],
                        start=st,
                        stop=stop,
                    )

            def t16(a, b, op, pool=tmp16, tg="tmp16"):
                o = pool.tile([P, T], F16, tag=tg, name=tg)
                nc.vector.tensor_tensor(out=o[0:pk, 0:fd], in0=a, in1=b, op=op)
                return o

            def act16(in_, func, bias=0.0, scale=1.0):
                o = tmp16.tile([P, T], F16, tag="tmp16", name="a16")
                nc.scalar.activation(
                    out=o[0:pk, 0:fd], in_=in_, func=func, bias=bias, scale=scale
                )
                return o

            adx1 = act16(trows["tx1"], AF.Abs, bias=cols["nqx1"])
            adx2 = act16(trows["tx2"], AF.Abs, bias=cols["nqx2"])
            uX = t16(adx1[0:pk, 0:fd], adx2[0:pk, 0:fd], OP.add, pool=long16, tg="long16")
            ady1 = act16(trows["ty1"], AF.Abs, bias=cols["nqy1"])
            ady2 = act16(trows["ty2"], AF.Abs, bias=cols["nqy2"])
            uY = t16(ady1[0:pk, 0:fd], ady2[0:pk, 0:fd], OP.add, pool=long16, tg="long16")

            # intersection x4
            sxw = t16(trows["Rw16"], uX[0:pk, 0:fd], OP.subtract)
            px = act16(sxw[0:pk, 0:fd], AF.Relu, bias=cols["qw"])
            syw = t16(trows["Rh16"], uY[0:pk, 0:fd], OP.subtract)
            py = act16(syw[0:pk, 0:fd], AF.Relu, bias=cols["qh"])
            inter4 = t16(px[0:pk, 0:fd], py[0:pk, 0:fd], OP.mult)

            # -(4 union + 4 eps); iou
            nun = tmp32.tile([P, T], F32, tag="tmp32", name="nun")
            nc.vector.scalar_tensor_tensor(
                out=nun[0:pk, 0:fd],
                in0=inter4[0:pk, 0:fd],
                scalar=cols["qa4e"],
                in1=trows["Ra4"],
                op0=OP.subtract,
                op1=OP.subtract,
            )
            rnu = tmp32.tile([P, T], F32, tag="tmp32", name="rnu")
            nc.vector.reciprocal_approx_fast(out=rnu[0:pk, 0:fd], in_=nun[0:pk, 0:fd])
            niou = add16.tile([P, T], F16, tag="add16", name="niou")  # -iou
            nc.vector.tensor_tensor(
                out=niou[0:pk, 0:fd],
                in0=inter4[0:pk, 0:fd],
                in1=rnu[0:pk, 0:fd],
                op=OP.mult,
            )
            accum(niou, stop=False)

            # convex diag x4
            cwx = t16(trows["Rw16"], uX[0:pk, 0:fd], OP.add)
            sqcw = act16(cwx[0:pk, 0:fd], AF.Square, bias=cols["qw"])
            cwy = t16(trows["Rh16"], uY[0:pk, 0:fd], OP.add)
            sqch = act16(cwy[0:pk, 0:fd], AF.Square, bias=cols["qh"])
            diag = tmp32.tile([P, T], F32, tag="tmp32", name="diag")
            nc.vector.scalar_tensor_tensor(
                out=diag[0:pk, 0:fd],
                in0=sqcw[0:pk, 0:fd],
                scalar=4.0 * EPS,
                in1=sqch[0:pk, 0:fd],
                op0=OP.add,
                op1=OP.add,
            )
            rd = tmp32.tile([P, T], F32, tag="tmp32", name="rd")
            nc.vector.reciprocal_approx_fast(out=rd[0:pk, 0:fd], in_=diag[0:pk, 0:fd])

            # center distance
            ex = act16(trows["Rcx"], AF.Square, bias=cols["nqcx"])
            ey = act16(trows["Rcy"], AF.Square, bias=cols["nqcy"])
            cd4 = t16(ex[0:pk, 0:fd], ey[0:pk, 0:fd], OP.add)
            pen = add16.tile([P, T], F16, tag="add16", name="pen")
            nc.vector.tensor_tensor(
                out=pen[0:pk, 0:fd], in0=cd4[0:pk, 0:fd], in1=rd[0:pk, 0:fd], op=OP.mult
            )
            accum(pen, stop=False)

            # v and alpha*v
            v = act16(trows["Rat"], AF.Square, bias=cols["nqat"], scale=2.0 / math.pi)
            aden = tmp32.tile([P, T], F32, tag="tmp32", name="aden")
            nc.vector.scalar_tensor_tensor(
                out=aden[0:pk, 0:fd],
                in0=niou[0:pk, 0:fd],
                scalar=1.0 + EPS,
                in1=v[0:pk, 0:fd],
                op0=OP.add,
                op1=OP.add,
            )
            ra = tmp32.tile([P, T], F32, tag="tmp32", name="ra")
            nc.vector.reciprocal_approx_fast(out=ra[0:pk, 0:fd], in_=aden[0:pk, 0:fd])
            vsq = act16(v[0:pk, 0:fd], AF.Square)
            av = add16.tile([P, T], F16, tag="add16", name="av")
            nc.vector.tensor_tensor(
                out=av[0:pk, 0:fd], in0=vsq[0:pk, 0:fd], in1=ra[0:pk, 0:fd], op=OP.mult
            )
            accum(av, stop=False)

            # L1
            L1 = add16.tile([P, T], F16, tag="add16", name="L1")
            nc.vector.tensor_tensor(
                out=L1[0:pk, 0:fd], in0=uX[0:pk, 0:fd], in1=uY[0:pk, 0:fd], op=OP.add
            )
            accum(L1, stop=True)

        # -------- 7 full query tiles --------
        for k in [kk for _rep in range(REPEAT) for kk in range(NFULL)]:
            pk = P
            sl = slice(k, k + 1)
            g = gpsum.tile([P, T], F32, tag="g", name="g")
            for n0, n1 in N_CHUNKS:
                nc.tensor.matmul(
                    g[0:pk, n0:n1],
                    lhsT=eT[:, k, 0:pk],
                    rhs=onehot16[:, n0:n1],
                    start=True,
                    stop=False,
                )
            cols = {
                "qw": qw8[0:pk, sl],
                "qh": qh8[0:pk, sl],
                "qa4e": qa4e8[0:pk, sl],
                "nqx1": nqx1_8[0:pk, sl],
                "nqy1": nqy1_8[0:pk, sl],
                "nqx2": nqx2_8[0:pk, sl],
                "nqy2": nqy2_8[0:pk, sl],
                "nqcx": nqcx8[0:pk, sl],
                "nqcy": nqcy8[0:pk, sl],
                "nqat": nqat8[0:pk, sl],
            }
            trows = {
                "tx1": tx1b[0:pk, :],
                "ty1": ty1b[0:pk, :],
                "tx2": tx2b[0:pk, :],
                "ty2": ty2b[0:pk, :],
                "Rw16": RwH[0:pk, :],
                "Rh16": RhH[0:pk, :],
                "Ra4": Ra4[0:pk, :],
                "Rcx": Rcx[0:pk, :],
                "Rcy": Rcy[0:pk, :],
                "Rat": Rat[0:pk, :],
            }
            emit_dag(pk, T, g, N_CHUNKS, cols, trows, class_starts=True)
            ost = ostage.tile([P, T], F32, tag="ostage", name="ost")
            nc.scalar.copy(out=ost[0:pk, :], in_=g[0:pk, :])
            nc.sync.dma_start(out=out_h[k * P : k * P + pk, :], in_=ost[0:pk, :])

        # -------- repacked tail: 4 queries x 1600 targets as [128, 50] --------
        # partition p = q*32 + c: query 896+q, target window [50c, 50c+50)

        for q in range(TQ):
            nc.sync.dma_start(
                out=tqb[q * TC : (q + 1) * TC, :],
                in_=bass.AP(
                    tensor=qbox_h[:, :].tensor,
                    offset=qbox_h[:, :].offset + (q0 + q) * 4,
                    ap=[[0, TC], [1, 4]],
                ),
            )

        nc.vector.tensor_tensor(out=tqw, in0=tqb[:, 2:3], in1=tqb[:, 0:1], op=OP.subtract)
        nc.vector.tensor_tensor(out=tqh, in0=tqb[:, 3:4], in1=tqb[:, 1:2], op=OP.subtract)
        for dst, src in (
            (tnqx1, tqb[:, 0:1]),
            (tnqy1, tqb[:, 1:2]),
            (tnqx2, tqb[:, 2:3]),
            (tnqy2, tqb[:, 3:4]),
        ):
            nc.vector.tensor_scalar(out=dst, in0=src, scalar1=-1.0, scalar2=None, op0=OP.mult)
        nc.vector.scalar_tensor_tensor(
            out=tqa4e, in0=tqw, scalar=4.0, in1=tqh, op0=OP.mult, op1=OP.mult
        )
        nc.vector.tensor_scalar(
            out=tqa4e, in0=tqa4e, scalar1=4.0 * EPS, scalar2=None, op0=OP.add
        )
        nc.vector.scalar_tensor_tensor(
            out=tnqcx, in0=tqb[:, 0:1], scalar=-1.0, in1=tqb[:, 2:3], op0=OP.mult, op1=OP.subtract
        )
        nc.vector.scalar_tensor_tensor(
            out=tnqcy, in0=tqb[:, 1:2], scalar=-1.0, in1=tqb[:, 3:4], op0=OP.mult, op1=OP.subtract
        )
        _ttc = [0]

        def _mkt1():
            t = _tat_tiles[_ttc[0]]
            _ttc[0] += 1
            return t

        emit_atan(tqat, tqw, tqh, _mkt1)
        nc.vector.tensor_scalar(
            out=tnqat, in0=tqat, scalar1=-2.0 / math.pi, scalar2=None, op0=OP.mult
        )


        # tail target rows in repacked layout (from DRAM tbox)
        for q in range(TQ):
            nc.sync.dma_start(
                out=trawt[q * TC : (q + 1) * TC, :, :],
                in_=bass.AP(
                    tensor=tbox_h[:, :].tensor,
                    offset=tbox_h[:, :].offset,
                    ap=[[TW * 4, TC], [4, TW], [1, 4]],
                ),
            )
        ttx1 = trawt[:, :, 0]
        tty1 = trawt[:, :, 1]
        ttx2 = trawt[:, :, 2]
        tty2 = trawt[:, :, 3]
        nc.vector.tensor_tensor(out=Rw32t, in0=ttx2, in1=ttx1, op=OP.subtract)
        nc.vector.tensor_tensor(out=Rh32t, in0=tty2, in1=tty1, op=OP.subtract)
        nc.vector.tensor_copy(RwHt[:, :], Rw32t[:, :])
        nc.vector.tensor_copy(RhHt[:, :], Rh32t[:, :])
        nc.vector.scalar_tensor_tensor(
            out=Ra4t, in0=Rw32t, scalar=4.0, in1=Rh32t, op0=OP.mult, op1=OP.mult
        )
        nc.vector.tensor_tensor(out=Rcxt, in0=ttx1, in1=ttx2, op=OP.add)
        nc.vector.tensor_tensor(out=Rcyt, in0=tty1, in1=tty2, op=OP.add)
        _ttc2 = [0]

        def _mkt2():
            _ttc2[0] += 1
            t = tmp32.tile([P, T], F32, tag="tmp32", name="tatw")
            return t[0:P, 0:TW]

        emit_atan(Ratt, Rw32t, Rh32t, _mkt2)

        # tail class term: matmul in [4, 1600], copy out, reshape to [128, 50]
        g4 = gpsum.tile([P, T], F32, tag="g", name="g4")
        for n0, n1 in N_CHUNKS:
            nc.tensor.matmul(
                g4[0:TQ, n0:n1],
                lhsT=eT[:, NFULL, 0:TQ],
                rhs=onehot16[:, n0:n1],
                start=True,
                stop=True,
            )
        gst = ostage.tile([P, T], F32, tag="ostage", name="gst")
        nc.scalar.copy(out=gst[0:TQ, :], in_=g4[0:TQ, :])
        gdram = nc.dram_tensor("tail_g", [TQ, T], F32)
        nc.sync.dma_start(out=gdram[:, :], in_=gst[0:TQ, :])
        g50 = tmp32.tile([P, T], F32, tag="tmp32", name="g50")
        for q in range(TQ):
            nc.sync.dma_start(
                out=g50[q * TC : (q + 1) * TC, 0:TW],
                in_=bass.AP(
                    tensor=gdram[:, :].tensor,
                    offset=gdram[:, :].offset + q * T,
                    ap=[[TW, TC], [1, TW]],
                ),
            )

        # tail DAG
        gt = gpsum.tile([P, T], F32, tag="g", name="gt")
        tcols = {
            "qw": tqw,
            "qh": tqh,
            "qa4e": tqa4e,
            "nqx1": tnqx1,
            "nqy1": tnqy1,
            "nqx2": tnqx2,
            "nqy2": tnqy2,
            "nqcx": tnqcx,
            "nqcy": tnqcy,
            "nqat": tnqat,
        }
        ttrows = {
            "tx1": ttx1,
            "ty1": tty1,
            "tx2": ttx2,
            "ty2": tty2,
            "Rw16": RwHt[:, :],
            "Rh16": RhHt[:, :],
            "Ra4": Ra4t[:, :],
            "Rcx": Rcxt[:, :],
            "Rcy": Rcyt[:, :],
            "Rat": Ratt[:, :],
        }
        emit_dag(P, TW, gt, [(0, TW)], tcols, ttrows, class_starts=False)

        ostt = ostage.tile([P, T], F32, tag="ostage", name="ostt")
        nc.vector.tensor_tensor(
            out=ostt[:, 0:TW], in0=g50[:, 0:TW], in1=gt[:, 0:TW], op=OP.add
        )
        for q in range(TQ):
            nc.sync.dma_start(
                out=bass.AP(
                    tensor=out_h[:, :].tensor,
                    offset=out_h[:, :].offset + (q0 + q) * T,
                    ap=[[TW, TC], [1, TW]],
                ),
                in_=ostt[q * TC : (q + 1) * TC, 0:TW],
            )

    nc.compile()
    return nc


_NC_CACHE = None


def _get_nc():
    global _NC_CACHE
    if _NC_CACHE is None:
        _NC_CACHE = build_kernel()
    return _NC_CACHE


def kernel(pred_logits, pred_bbox, tgt_ids, tgt_bbox, **_unused):
    pred_logits = np.ascontiguousarray(np.asarray(pred_logits, dtype=np.float32))
    pred_bbox = np.ascontiguousarray(np.asarray(pred_bbox, dtype=np.float32))
    tgt_bbox = np.ascontiguousarray(np.asarray(tgt_bbox, dtype=np.float32))
    tid = np.ascontiguousarray(np.asarray(tgt_ids).astype(np.int32))

    nc = _get_nc()
    in_maps = [
        {
            "logits": pred_logits[i],
            "qbox": pred_bbox[i],
            "tbox": tgt_bbox,
            "tid": tid,
        }
        for i in range(B)
    ]
    res = run_bass_kernel_spmd(nc, in_maps, list(range(B)))
    out = np.stack([res.results[i]["out"] for i in range(B)], axis=0)
    return out.astype(np.float32)


if __name__ == "__main__":
    nc = build_kernel()
    print("v2 built OK")


# revision 13
# speedup vs baseline: 27.9034x; 6.0382x over previous
"""Trainium2 Bass kernel v2: fp16 interior + TensorE-accumulated final sums.

cost[b, q, t] = L1(pred_box, tgt_box) - softmax(logits)[q, tgt_id[t]] - CIoU(pred_box, tgt_box)

Per-core (batch-parallel) plan, queries on partitions, targets on free dim.
The final cost = (-prob) + L1 + (-iou) + pen + alpha*v is accumulated in PSUM:
the class term via an fp16 matmul expT_scaled @ onehot, the four per-pair
addends via fp16 identity matmuls. fp32 is kept on the reciprocal chains.
"""

import math
from contextlib import ExitStack

import numpy as np

import concourse.bass as bass
import concourse.bacc as bacc
import concourse.mybir as mybir
import concourse.tile as tile
from concourse.bass_utils import run_bass_kernel_spmd
from concourse.masks import make_identity

B, Q, C, T = 8, 900, 92, 1600
REPEAT = 1
EPS = 1e-6
P = 128
NQT = (Q + P - 1) // P  # 8 query tiles; last is ragged (4 rows)
F32 = mybir.dt.float32
F16 = mybir.dt.float16
I32 = mybir.dt.int32
AF = mybir.ActivationFunctionType
OP = mybir.AluOpType
AX = mybir.AxisListType

N_CHUNKS = [(0, 512), (512, 1024), (1024, 1536), (1536, 1600)]


def _bcast_ap(ap, npart, inner_ap):
    return bass.AP(tensor=ap.tensor, offset=ap.offset, ap=[[0, npart]] + inner_ap)


def build_kernel():
    nc = bacc.Bacc()

    logits_h = nc.declare_dram_parameter("logits", [Q, C], F32, isOutput=False)
    qbox_h = nc.declare_dram_parameter("qbox", [Q, 4], F32, isOutput=False)
    tbox_h = nc.declare_dram_parameter("tbox", [T, 4], F32, isOutput=False)
    tid_h = nc.declare_dram_parameter("tid", [T], I32, isOutput=False)
    out_h = nc.declare_dram_parameter("out", [Q, T], F32, isOutput=True)

    with ExitStack() as ctx:
        tc = ctx.enter_context(tile.TileContext(nc))
        consts = ctx.enter_context(tc.tile_pool(name="consts", bufs=1))
        rows = ctx.enter_context(tc.tile_pool(name="rows", bufs=1))
        qcols = ctx.enter_context(tc.tile_pool(name="qcols", bufs=1))

        # ---------------- constants ----------------
        ident_h = consts.tile([P, P], F16, tag="ident_h")
        make_identity(nc, ident_h)
        ic_i = consts.tile([C, 1], I32, tag="ic_i")
        nc.gpsimd.iota(ic_i, pattern=[[0, 1]], base=0, channel_multiplier=1)
        ic_f = consts.tile([C, 1], F32, tag="ic_f")
        nc.vector.tensor_copy(ic_f, ic_i)

        # persistent target rows
        traw = rows.tile([P, T, 4], F32, tag="traw")
        RwH = rows.tile([P, T], F16, tag="RwH")
        RhH = rows.tile([P, T], F16, tag="RhH")
        Ra4 = rows.tile([P, T], F32, tag="Ra4")
        Rcx = rows.tile([P, T], F32, tag="Rcx")
        Rcy = rows.tile([P, T], F32, tag="Rcy")
        Rat = rows.tile([P, T], F32, tag="Rat")
        onehot16 = rows.tile([C, T], F16, tag="onehot16")

        # tail tiles pre-allocated here so no new tags land in rows/qcols after
        # the scratch pool below is freed (Tile mis-syncs recycled regions)
        TQ, TC, TW = 4, 32, 50
        q0 = Q - TQ  # 896
        NFULL = NQT - 1
        trawt = rows.tile([P, TW, 4], F32, tag="trawt")
        Rw32t = rows.tile([P, TW], F32, tag="Rw32t")
        Rh32t = rows.tile([P, TW], F32, tag="Rh32t")
        RwHt = rows.tile([P, TW], F16, tag="RwHt")
        RhHt = rows.tile([P, TW], F16, tag="RhHt")
        Ra4t = rows.tile([P, TW], F32, tag="Ra4t")
        Rcxt = rows.tile([P, TW], F32, tag="Rcxt")
        Rcyt = rows.tile([P, TW], F32, tag="Rcyt")
        Ratt = rows.tile([P, TW], F32, tag="Ratt")
        tqb = qcols.tile([P, 4], F32, tag="tqb")

        def ttile(tag):
            return qcols.tile([P, 1], F32, tag=tag, name=tag)

        tqw = ttile("tqw")
        tqh = ttile("tqh")
        tnqx1 = ttile("tnqx1")
        tnqy1 = ttile("tnqy1")
        tnqx2 = ttile("tnqx2")
        tnqy2 = ttile("tnqy2")
        tqa4e = ttile("tqa4e")
        tnqcx = ttile("tnqcx")
        tnqcy = ttile("tnqcy")
        tqat = ttile("tqat")
        tnqat = ttile("tnqat")
        _tat_tiles = [ttile(f"tat{i}") for i in range(9)]

        # broadcast raw tbox [1600,4] to all partitions (doubling DMA)
        nc.sync.dma_start(
            out=traw[:, :, :], in_=_bcast_ap(tbox_h[:, :], P, [[4, T], [1, 4]])
        )

        tx1b = traw[:, :, 0]
        ty1b = traw[:, :, 1]
        tx2b = traw[:, :, 2]
        ty2b = traw[:, :, 3]

        # arctan with range reduction: atan(r) = pi/2 - atan(1/r) for r > 1
        def emit_atan(dst, wt, ht, mkt):
            t1 = mkt()
            nc.vector.tensor_scalar(
                out=t1, in0=ht, scalar1=EPS, scalar2=None, op0=OP.add
            )
            t2 = mkt()
            nc.vector.reciprocal_approx_fast(out=t2, in_=t1)
            r = mkt()
            nc.vector.tensor_tensor(out=r, in0=wt, in1=t2, op=OP.mult)
            ri = mkt()
            nc.vector.reciprocal_approx_fast(out=ri, in_=r)
            rc = mkt()
            nc.vector.tensor_tensor(out=rc, in0=r, in1=ri, op=OP.min)
            atc = mkt()
            nc.scalar.activation(out=atc, in_=rc, func=AF.Arctan)
            m = mkt()
            nc.vector.tensor_scalar(
                out=m, in0=r, scalar1=1.0, scalar2=None, op0=OP.is_gt
            )
            t3 = mkt()
            nc.vector.tensor_scalar(
                out=t3,
                in0=atc,
                scalar1=-2.0,
                scalar2=math.pi / 2.0,
                op0=OP.mult,
                op1=OP.add,
            )
            mt = mkt()
            nc.vector.tensor_tensor(out=mt, in0=m, in1=t3, op=OP.mult)
            nc.vector.tensor_tensor(out=dst, in0=atc, in1=mt, op=OP.add)

        # scratch pool: freed before the main loop pools open
        with tc.tile_pool(name="scratch", bufs=1) as scratch:
            # one-hot from tgt ids
            tid_i = scratch.tile([C, T], I32, tag="tid_i")
            nc.sync.dma_start(out=tid_i[:, :], in_=_bcast_ap(tid_h[:], C, [[1, T]]))
            tid_f = scratch.tile([C, T], F32, tag="tid_f")
            nc.vector.tensor_copy(tid_f[:, :], tid_i[:, :])
            oh32 = scratch.tile([C, T], F32, tag="oh32")
            nc.vector.tensor_scalar(
                out=oh32, in0=tid_f, scalar1=ic_f[:, 0:1], scalar2=None, op0=OP.is_equal
            )
            nc.vector.tensor_copy(onehot16[:, :], oh32[:, :])

            Rw = scratch.tile([P, T], F32, tag="Rw")
            Rh = scratch.tile([P, T], F32, tag="Rh")
            nc.vector.tensor_tensor(out=Rw, in0=tx2b, in1=tx1b, op=OP.subtract)
            nc.vector.tensor_tensor(out=Rh, in0=ty2b, in1=ty1b, op=OP.subtract)
            nc.vector.tensor_copy(RwH[:, :], Rw[:, :])
            nc.vector.tensor_copy(RhH[:, :], Rh[:, :])
            nc.vector.scalar_tensor_tensor(
                out=Ra4, in0=Rw, scalar=4.0, in1=Rh, op0=OP.mult, op1=OP.mult
            )
            nc.vector.tensor_tensor(out=Rcx, in0=tx1b, in1=tx2b, op=OP.add)
            nc.vector.tensor_tensor(out=Rcy, in0=ty1b, in1=ty2b, op=OP.add)

            _atc = [0]

            def _mka():
                _atc[0] += 1
                return scratch.tile([P, T], F32, tag="att", name="att", bufs=5)

            emit_atan(Rat, Rw, Rh, _mka)

            # ------------- per-query columns (inside scratch epoch is fine; they
            # live in qcols which persists) -------------
            qb = qcols.tile([P, NQT, 4], F32, tag="qb")
            nc.vector.memset(qb, 1.0)
            nc.vector.memset(qb[:, :, 0:2], 0.25)
            nfull = Q // P
            nc.sync.dma_start(
                out=qb[:, 0:nfull, :],
                in_=bass.AP(
                    tensor=qbox_h[:, :].tensor,
                    offset=qbox_h[:, :].offset,
                    ap=[[4, P], [P * 4, nfull], [1, 4]],
                ),
            )
            nc.sync.dma_start(out=qb[0 : Q - nfull * P, nfull, :], in_=qbox_h[nfull * P : Q, :])

            qx1 = qb[:, :, 0]
            qy1 = qb[:, :, 1]
            qx2 = qb[:, :, 2]
            qy2 = qb[:, :, 3]

            def qtile(tag):
                return qcols.tile([P, NQT], F32, tag=tag, name=tag)

            qw8 = qtile("qw8")
            qh8 = qtile("qh8")
            nc.vector.tensor_tensor(out=qw8, in0=qx2, in1=qx1, op=OP.subtract)
            nc.vector.tensor_tensor(out=qh8, in0=qy2, in1=qy1, op=OP.subtract)
            nqx1_8 = qtile("nqx1")
            nqy1_8 = qtile("nqy1")
            nqx2_8 = qtile("nqx2")
            nqy2_8 = qtile("nqy2")
            for dst, src in (
                (nqx1_8, qx1),
                (nqy1_8, qy1),
                (nqx2_8, qx2),
                (nqy2_8, qy2),
            ):
                nc.vector.tensor_scalar(
                    out=dst, in0=src, scalar1=-1.0, scalar2=None, op0=OP.mult
                )
            qa4e8 = qtile("qa4e")
            nc.vector.scalar_tensor_tensor(
                out=qa4e8, in0=qw8, scalar=4.0, in1=qh8, op0=OP.mult, op1=OP.mult
            )
            nc.vector.tensor_scalar(
                out=qa4e8, in0=qa4e8, scalar1=4.0 * EPS, scalar2=None, op0=OP.add
            )
            nqcx8 = qtile("nqcx")
            nqcy8 = qtile("nqcy")
            nc.vector.scalar_tensor_tensor(
                out=nqcx8, in0=qx1, scalar=-1.0, in1=qx2, op0=OP.mult, op1=OP.subtract
            )
            nc.vector.scalar_tensor_tensor(
                out=nqcy8, in0=qy1, scalar=-1.0, in1=qy2, op0=OP.mult, op1=OP.subtract
            )
            qat = qtile("qat")
            _qtc = [0]

            def _mkq():
                _qtc[0] += 1
                return qcols.tile([P, NQT], F32, tag=f"qat_t{_qtc[0]}", name="qat_t")

            emit_atan(qat, qw8, qh8, _mkq)
            nqat8 = qtile("nqat")
            nc.vector.tensor_scalar(
                out=nqat8, in0=qat, scalar1=-2.0 / math.pi, scalar2=None, op0=OP.mult
            )

            # ------------- softmax (phase A): exp + row sums -------------
            mneg8 = qcols.tile([P, NQT], F32, tag="mneg8")
            ssum8 = qcols.tile([P, NQT], F32, tag="ssum8")
            nc.vector.memset(ssum8, 1.0)
            e_all = qcols.tile([P, NQT, C], F32, tag="e_all")

            for k in range(NQT):
                pk = min(P, Q - k * P)
                L = scratch.tile([P, C], F32, tag="L", name="L", bufs=3)
                nc.sync.dma_start(
                    out=L[0:pk, :], in_=logits_h[k * P : k * P + pk, :]
                )
                nc.vector.tensor_reduce(
                    out=mneg8[0:pk, k : k + 1],
                    in_=L[0:pk, :],
                    axis=AX.X,
                    op=OP.max,
                    negate=True,
                )
                nc.scalar.activation(
                    out=e_all[0:pk, k, :],
                    in_=L[0:pk, :],
                    func=AF.Exp,
                    bias=mneg8[0:pk, k : k + 1],
                    scale=1.0,
                    accum_out=ssum8[0:pk, k : k + 1],
                )

            # nr = -1/sum(exp)
            nr8 = qcols.tile([P, NQT], F32, tag="nr8")
            nc.vector.reciprocal(out=nr8, in_=ssum8)
            nc.vector.tensor_scalar(
                out=nr8, in0=nr8, scalar1=-1.0, scalar2=None, op0=OP.mult
            )

        # ------------- softmax (phase B): scale by -1/sum, transpose (fp16) ----
        eT = qcols.tile([C, NQT, P], F16, tag="eT")
        with tc.tile_pool(name="tposep", bufs=2, space="PSUM") as tpsum, tc.tile_pool(
            name="es16", bufs=2
        ) as es16:
            for k in range(NQT):
                pk = min(P, Q - k * P)
                es = es16.tile([P, C], F16, tag="es", name="es")
                nc.vector.tensor_scalar(
                    out=es[0:pk, :],
                    in0=e_all[0:pk, k, :],
                    scalar1=nr8[0:pk, k : k + 1],
                    scalar2=None,
                    op0=OP.mult,
                )
                tp = tpsum.tile([C, P], F16, tag="tp", name="tp")
                nc.tensor.transpose(tp[:, 0:pk], es[0:pk, :], ident_h[0:pk, 0:pk])
                nc.scalar.copy(out=eT[:, k, 0:pk], in_=tp[:, 0:pk])

        # ---------------- main loop pools ----------------
        long16 = ctx.enter_context(tc.tile_pool(name="long16", bufs=6))
        add16 = ctx.enter_context(tc.tile_pool(name="add16", bufs=8))
        tmp16 = ctx.enter_context(tc.tile_pool(name="tmp16", bufs=12))
        tmp32 = ctx.enter_context(tc.tile_pool(name="tmp32", bufs=7))
        ostage = ctx.enter_context(tc.tile_pool(name="ostage", bufs=2))
        gpsum = ctx.enter_context(tc.tile_pool(name="gpsum", bufs=2, space="PSUM"))

        def emit_dag(pk, fd, g, chunks, cols, trows, class_starts):
            """Emit the per-pair cost DAG into PSUM tile `g` ([pk, fd] region).

            cols: per-query [pk,1] APs; trows: target-row APs at [pk, fd].
            If class_starts, the class matmuls already started the PSUM group.
            """
            first = [not class_starts]

            def accum(x, stop):
                st = first[0]
                first[0] = False
                for n0, n1 in chunks:
                    nc.tensor.matmul(
                        g[0:pk, n0:n1],
                        lhsT=ident_h[0:pk, 0:pk],
                        rhs=x[0:pk, n0:n1],
                        start=st,
                        stop=stop,
                    )

            def t16(a, b, op, pool=tmp16, tg="tmp16"):
                o = pool.tile([P, T], F16, tag=tg, name=tg)
                nc.vector.tensor_tensor(out=o[0:pk, 0:fd], in0=a, in1=b, op=op)
                return o

            def act16(in_, func, bias=0.0, scale=1.0):
                o = tmp16.tile([P, T], F16, tag="tmp16", name="a16")
                nc.scalar.activation(
                    out=o[0:pk, 0:fd], in_=in_, func=func, bias=bias, scale=scale
                )
                return o

            adx1 = act16(trows["tx1"], AF.Abs, bias=cols["nqx1"])
            adx2 = act16(trows["tx2"], AF.Abs, bias=cols["nqx2"])
            uX = t16(adx1[0:pk, 0:fd], adx2[0:pk, 0:fd], OP.add, pool=long16, tg="long16")
            ady1 = act16(trows["ty1"], AF.Abs, bias=cols["nqy1"])
            ady2 = act16(trows["ty2"], AF.Abs, bias=cols["nqy2"])
            uY = t16(ady1[0:pk, 0:fd], ady2[0:pk, 0:fd], OP.add, pool=long16, tg="long16")

            # intersection x4
            sxw = t16(trows["Rw16"], uX[0:pk, 0:fd], OP.subtract)
            px = act16(sxw[0:pk, 0:fd], AF.Relu, bias=cols["qw"])
            syw = t16(trows["Rh16"], uY[0:pk, 0:fd], OP.subtract)
            py = act16(syw[0:pk, 0:fd], AF.Relu, bias=cols["qh"])
            inter4 = t16(px[0:pk, 0:fd], py[0:pk, 0:fd], OP.mult)

            # -(4 union + 4 eps); iou
            nun = tmp32.tile([P, T], F32, tag="tmp32", name="nun")
            nc.vector.scalar_tensor_tensor(
                out=nun[0:pk, 0:fd],
                in0=inter4[0:pk, 0:fd],
                scalar=cols["qa4e"],
                in1=trows["Ra4"],
                op0=OP.subtract,
                op1=OP.subtract,
            )
            rnu = tmp32.tile([P, T], F32, tag="tmp32", name="rnu")
            nc.vector.reciprocal_approx_fast(out=rnu[0:pk, 0:fd], in_=nun[0:pk, 0:fd])
            niou = add16.tile([P, T], F16, tag="add16", name="niou")  # -iou
            nc.vector.tensor_tensor(
                out=niou[0:pk, 0:fd],
                in0=inter4[0:pk, 0:fd],
                in1=rnu[0:pk, 0:fd],
                op=OP.mult,
            )
            accum(niou, stop=False)

            # convex diag x4
            cwx = t16(trows["Rw16"], uX[0:pk, 0:fd], OP.add)
            sqcw = act16(cwx[0:pk, 0:fd], AF.Square, bias=cols["qw"])
            cwy = t16(trows["Rh16"], uY[0:pk, 0:fd], OP.add)
            sqch = act16(cwy[0:pk, 0:fd], AF.Square, bias=cols["qh"])
            diag = tmp32.tile([P, T], F32, tag="tmp32", name="diag")
            nc.vector.scalar_tensor_tensor(
                out=diag[0:pk, 0:fd],
                in0=sqcw[0:pk, 0:fd],
                scalar=4.0 * EPS,
                in1=sqch[0:pk, 0:fd],
                op0=OP.add,
                op1=OP.add,
            )
            rd = tmp32.tile([P, T], F32, tag="tmp32", name="rd")
            nc.vector.reciprocal_approx_fast(out=rd[0:pk, 0:fd], in_=diag[0:pk, 0:fd])

            # center distance
            ex = act16(trows["Rcx"], AF.Square, bias=cols["nqcx"])
            ey = act16(trows["Rcy"], AF.Square, bias=cols["nqcy"])
            cd4 = t16(ex[0:pk, 0:fd], ey[0:pk, 0:fd], OP.add)
            pen = add16.tile([P, T], F16, tag="add16", name="pen")
            nc.vector.tensor_tensor(
                out=pen[0:pk, 0:fd], in0=cd4[0:pk, 0:fd], in1=rd[0:pk, 0:fd], op=OP.mult
            )
            accum(pen, stop=False)

            # v and alpha*v
            v = act16(trows["Rat"], AF.Square, bias=cols["nqat"], scale=2.0 / math.pi)
            aden = tmp32.tile([P, T], F32, tag="tmp32", name="aden")
            nc.vector.scalar_tensor_tensor(
                out=aden[0:pk, 0:fd],
                in0=niou[0:pk, 0:fd],
                scalar=1.0 + EPS,
                in1=v[0:pk, 0:fd],
                op0=OP.add,
                op1=OP.add,
            )
            ra = tmp32.tile([P, T], F32, tag="tmp32", name="ra")
            nc.vector.reciprocal_approx_fast(out=ra[0:pk, 0:fd], in_=aden[0:pk, 0:fd])
            vsq = act16(v[0:pk, 0:fd], AF.Square)
            av = add16.tile([P, T], F16, tag="add16", name="av")
            nc.vector.tensor_tensor(
                out=av[0:pk, 0:fd], in0=vsq[0:pk, 0:fd], in1=ra[0:pk, 0:fd], op=OP.mult
            )
            accum(av, stop=False)

            # L1
            L1 = add16.tile([P, T], F16, tag="add16", name="L1")
            nc.vector.tensor_tensor(
                out=L1[0:pk, 0:fd], in0=uX[0:pk, 0:fd], in1=uY[0:pk, 0:fd], op=OP.add
            )
            accum(L1, stop=True)

        # -------- 7 full query tiles --------
        for k in [kk for _rep in range(REPEAT) for kk in range(NFULL)]:
            pk = P
            sl = slice(k, k + 1)
            g = gpsum.tile([P, T], F32, tag="g", name="g")
            for n0, n1 in N_CHUNKS:
                nc.tensor.matmul(
                    g[0:pk, n0:n1],
                    lhsT=eT[:, k, 0:pk],
                    rhs=onehot16[:, n0:n1],
                    start=True,
                    stop=False,
                )
            cols = {
                "qw": qw8[0:pk, sl],
                "qh": qh8[0:pk, sl],
                "qa4e": qa4e8[0:pk, sl],
                "nqx1": nqx1_8[0:pk, sl],
                "nqy1": nqy1_8[0:pk, sl],
                "nqx2": nqx2_8[0:pk, sl],
                "nqy2": nqy2_8[0:pk, sl],
                "nqcx": nqcx8[0:pk, sl],
                "nqcy": nqcy8[0:pk, sl],
                "nqat": nqat8[0:pk, sl],
            }
            trows = {
                "tx1": tx1b[0:pk, :],
                "ty1": ty1b[0:pk, :],
                "tx2": tx2b[0:pk, :],
                "ty2": ty2b[0:pk, :],
                "Rw16": RwH[0:pk, :],
                "Rh16": RhH[0:pk, :],
                "Ra4": Ra4[0:pk, :],
                "Rcx": Rcx[0:pk, :],
                "Rcy": Rcy[0:pk, :],
                "Rat": Rat[0:pk, :],
            }
            emit_dag(pk, T, g, N_CHUNKS, cols, trows, class_starts=True)
            ost = ostage.tile([P, T], F32, tag="ostage", name="ost")
            nc.scalar.copy(out=ost[0:pk, :], in_=g[0:pk, :])
            nc.sync.dma_start(out=out_h[k * P : k * P + pk, :], in_=ost[0:pk, :])

        # -------- repacked tail: 4 queries x 1600 targets as [128, 50] --------
        # partition p = q*32 + c: query 896+q, target window [50c, 50c+50)

        for q in range(TQ):
            nc.sync.dma_start(
                out=tqb[q * TC : (q + 1) * TC, :],
                in_=bass.AP(
                    tensor=qbox_h[:, :].tensor,
                    offset=qbox_h[:, :].offset + (q0 + q) * 4,
                    ap=[[0, TC], [1, 4]],
                ),
            )

        nc.vector.tensor_tensor(out=tqw, in0=tqb[:, 2:3], in1=tqb[:, 0:1], op=OP.subtract)
        nc.vector.tensor_tensor(out=tqh, in0=tqb[:, 3:4], in1=tqb[:, 1:2], op=OP.subtract)
        for dst, src in (
            (tnqx1, tqb[:, 0:1]),
            (tnqy1, tqb[:, 1:2]),
            (tnqx2, tqb[:, 2:3]),
            (tnqy2, tqb[:, 3:4]),
        ):
            nc.vector.tensor_scalar(out=dst, in0=src, scalar1=-1.0, scalar2=None, op0=OP.mult)
        nc.vector.scalar_tensor_tensor(
            out=tqa4e, in0=tqw, scalar=4.0, in1=tqh, op0=OP.mult, op1=OP.mult
        )
        nc.vector.tensor_scalar(
            out=tqa4e, in0=tqa4e, scalar1=4.0 * EPS, scalar2=None, op0=OP.add
        )
        nc.vector.scalar_tensor_tensor(
            out=tnqcx, in0=tqb[:, 0:1], scalar=-1.0, in1=tqb[:, 2:3], op0=OP.mult, op1=OP.subtract
        )
        nc.vector.scalar_tensor_tensor(
            out=tnqcy, in0=tqb[:, 1:2], scalar=-1.0, in1=tqb[:, 3:4], op0=OP.mult, op1=OP.subtract
        )
        _ttc = [0]

        def _mkt1():
            t = _tat_tiles[_ttc[0]]
            _ttc[0] += 1
            return t

        emit_atan(tqat, tqw, tqh, _mkt1)
        nc.vector.tensor_scalar(
            out=tnqat, in0=tqat, scalar1=-2.0 / math.pi, scalar2=None, op0=OP.mult
        )


        # tail target rows in repacked layout (from DRAM tbox)
        for q in range(TQ):
            nc.sync.dma_start(
                out=trawt[q * TC : (q + 1) * TC, :, :],
                in_=bass.AP(
                    tensor=tbox_h[:, :].tensor,
                    offset=tbox_h[:, :].offset,
                    ap=[[TW * 4, TC], [4, TW], [1, 4]],
                ),
            )
        ttx1 = trawt[:, :, 0]
        tty1 = trawt[:, :, 1]
        ttx2 = trawt[:, :, 2]
        tty2 = trawt[:, :, 3]
        nc.vector.tensor_tensor(out=Rw32t, in0=ttx2, in1=ttx1, op=OP.subtract)
        nc.vector.tensor_tensor(out=Rh32t, in0=tty2, in1=tty1, op=OP.subtract)
        nc.vector.tensor_copy(RwHt[:, :], Rw32t[:, :])
        nc.vector.tensor_copy(RhHt[:, :], Rh32t[:, :])
        nc.vector.scalar_tensor_tensor(
            out=Ra4t, in0=Rw32t, scalar=4.0, in1=Rh32t, op0=OP.mult, op1=OP.mult
        )
        nc.vector.tensor_tensor(out=Rcxt, in0=ttx1, in1=ttx2, op=OP.add)
        nc.vector.tensor_tensor(out=Rcyt, in0=tty1, in1=tty2, op=OP.add)
        _ttc2 = [0]

        def _mkt2():
            _ttc2[0] += 1
            t = tmp32.tile([P, T], F32, tag="tmp32", name="tatw")
            return t[0:P, 0:TW]

        emit_atan(Ratt, Rw32t, Rh32t, _mkt2)

        # tail class term: matmul in [4, 1600], copy out, reshape to [128, 50]
        g4 = gpsum.tile([P, T], F32, tag="g", name="g4")
        for n0, n1 in N_CHUNKS:
            nc.tensor.matmul(
                g4[0:TQ, n0:n1],
                lhsT=eT[:, NFULL, 0:TQ],
                rhs=onehot16[:, n0:n1],
                start=True,
                stop=True,
            )
        gst = ostage.tile([P, T], F32, tag="ostage", name="gst")
        nc.scalar.copy(out=gst[0:TQ, :], in_=g4[0:TQ, :])
        gdram = nc.dram_tensor("tail_g", [TQ, T], F32)
        nc.sync.dma_start(out=gdram[:, :], in_=gst[0:TQ, :])
        g50 = tmp32.tile([P, T], F32, tag="tmp32", name="g50")
        for q in range(TQ):
            nc.sync.dma_start(
                out=g50[q * TC : (q + 1) * TC, 0:TW],
                in_=bass.AP(
                    tensor=gdram[:, :].tensor,
                    offset=gdram[:, :].offset + q * T,
                    ap=[[TW, TC], [1, TW]],
                ),
            )

        # tail DAG
        gt = gpsum.tile([P, T], F32, tag="g", name="gt")
        tcols = {
            "qw": tqw,
            "qh": tqh,
            "qa4e": tqa4e,
            "nqx1": tnqx1,
            "nqy1": tnqy1,
            "nqx2": tnqx2,
            "nqy2": tnqy2,
            "nqcx": tnqcx,
            "nqcy": tnqcy,
            "nqat": tnqat,
        }
        ttrows = {
            "tx1": ttx1,
            "ty1": tty1,
            "tx2": ttx2,
            "ty2": tty2,
            "Rw16": RwHt[:, :],
            "Rh16": RhHt[:, :],
            "Ra4": Ra4t[:, :],
            "Rcx": Rcxt[:, :],
            "Rcy": Rcyt[:, :],
            "Rat": Ratt[:, :],
        }
        emit_dag(P, TW, gt, [(0, TW)], tcols, ttrows, class_starts=False)

        ostt = ostage.tile([P, T], F32, tag="ostage", name="ostt")
        nc.vector.tensor_tensor(
            out=ostt[:, 0:TW], in0=g50[:, 0:TW], in1=gt[:, 0:TW], op=OP.add
        )
        for q in range(TQ):
            nc.sync.dma_start(
                out=bass.AP(
                    tensor=out_h[:, :].tensor,
                    offset=out_h[:, :].offset + (q0 + q) * T,
                    ap=[[TW, TC], [1, TW]],
                ),
                in_=ostt[q * TC : (q + 1) * TC, 0:TW],
            )

    nc.compile()
    return nc


_NC_CACHE = None


def _get_nc():
    global _NC_CACHE
    if _NC_CACHE is None:
        _NC_CACHE = build_kernel()
    return _NC_CACHE


def kernel(pred_logits, pred_bbox, tgt_ids, tgt_bbox, **_unused):
    pred_logits = np.ascontiguousarray(np.asarray(pred_logits, dtype=np.float32))
    pred_bbox = np.ascontiguousarray(np.asarray(pred_bbox, dtype=np.float32))
    tgt_bbox = np.ascontiguousarray(np.asarray(tgt_bbox, dtype=np.float32))
    tid = np.ascontiguousarray(np.asarray(tgt_ids).astype(np.int32))

    nc = _get_nc()
    in_maps = [
        {
            "logits": pred_logits[i],
            "qbox": pred_bbox[i],
            "tbox": tgt_bbox,
            "tid": tid,
        }
        for i in range(B)
    ]
    res = run_bass_kernel_spmd(nc, in_maps, list(range(B)))
    out = np.stack([res.results[i]["out"] for i in range(B)], axis=0)
    return out.astype(np.float32)


if __name__ == "__main__":
    nc = build_kernel()
    print("v2 built OK")
